# revision 6
# baseline (speedup 1.0000x reference)
"""Tensor-parallel fused attention kernel for Trainium2 (8 NeuronCores).

Sharding: DP=2 over batch x TP=4 over kv-head pairs. Each core computes
q/k/v projections + RoPE + causal attention + output projection for its
(batch, 2 kv heads) shard in bf16, then a 4-core ReduceScatter combines
the partial output projections; the host assembles the disjoint row
shards into the full [2, 2048, 4096] output.

Attention is computed in transposed-score layout: sT[kv, q] comes
straight out of matmul(lhsT=kT_j, rhs=qT), exp(sT) feeds the yT
accumulation directly (no per-block transpose matmuls), softmax row
sums come from an accumulating ones-vector matmul over the saved
exp tiles, and 1/Z is broadcast with a rank-1 matmul and folded into
the yT PSUM evacuation.
"""
import sys

for _p in ("/opt/trn_rl_repo", "/root/.axon_site/_ro/trn_rl_repo"):
    if _p not in sys.path:
        sys.path.append(_p)

import math
import numpy as np
import ml_dtypes

import concourse.bass as bass
import concourse.mybir as mybir
import concourse.tile as tile
from concourse import bacc
from concourse import bass_utils

BF16 = ml_dtypes.bfloat16
FP32 = mybir.dt.float32
BF = mybir.dt.bfloat16

B, S, D = 2, 2048, 4096
R, K, H = 4, 8, 128
N_CORES = 8
TP = 4            # tensor-parallel ways (kv-head axis)
KLOC = K // TP    # kv heads per core = 2
HEADS = R * KLOC  # query heads per core = 8
DT = D // 128     # 32 d-tiles
ST = S // 128     # 16 s-tiles
NG = ST // 4      # 4 strips of 512 rows
# ReduceScatter chunks: (start_tile, n_tiles); last two are single-tile to
# shorten the serial tail after the final out-projection.
CC_CHUNKS = [(0, 2), (2, 2), (4, 2), (6, 2), (8, 2), (10, 2), (12, 2), (14, 1), (15, 1)]

_CACHE = {}


def _build(causal: bool):
    nc = bacc.Bacc("TRN2", target_bir_lowering=False, debug=False,
                   enable_asserts=False, num_devices=N_CORES)

    xP = nc.dram_tensor("xP", [128, 2 * DT * (S // 2)], BF, kind="ExternalInput")
    wq = nc.dram_tensor("wq", [HEADS * 128, DT * 128], BF, kind="ExternalInput")
    wk = nc.dram_tensor("wk", [KLOC * 128, DT * 128], BF, kind="ExternalInput")
    wv = nc.dram_tensor("wv", [128, DT * KLOC * H], BF, kind="ExternalInput")
    wo = nc.dram_tensor("wo", [HEADS * H, D], BF, kind="ExternalInput")
    cosT = nc.dram_tensor("cosT", [H, S], FP32, kind="ExternalInput")
    sinST = nc.dram_tensor("sinST", [H, S], FP32, kind="ExternalInput")
    if causal:
        mdT = nc.dram_tensor("mdT", [128, S], FP32, kind="ExternalInput")
    else:
        maskTf = nc.dram_tensor("maskTf", [S, S], FP32, kind="ExternalInput")
    out_sh = nc.dram_tensor("out_shard", [S // TP, D], BF, kind="ExternalOutput")

    with tile.TileContext(nc) as tc:
        with tc.tile_pool(name="persist", bufs=1) as persist, \
             tc.tile_pool(name="dram", bufs=1, space="DRAM") as dram:

            kT_t = [persist.tile([128, S], BF, tag=f"kT{i}", name=f"kT{i}")
                    for i in range(KLOC)]
            v_t = [persist.tile([128, KLOC * H], BF, tag=f"v{i}", name=f"v{i}")
                   for i in range(ST)]
            wo_sb = [persist.tile([128, D], BF, tag=f"wo{i}", name=f"wo{i}")
                     for i in range(HEADS)]
            qT_dram = dram.tile([HEADS * 128, S], BF, tag="qtd", name="qT_dram")
            cc_in = [dram.tile([n * 128, D], BF, tag=f"ccin{g}", name=f"cc_in{g}")
                     for g, (st0, n) in enumerate(CC_CHUNKS)]
            cc_out = [dram.tile([n * 32, D], BF, tag=f"ccout{g}", name=f"cc_out{g}")
                      for g, (st0, n) in enumerate(CC_CHUNKS)]

            # ---------------- Phase 1: projections + rope ----------------
            with tc.tile_pool(name="p1", bufs=1) as p1, \
                 tc.tile_pool(name="p1ps", bufs=1, space="PSUM") as p1ps:
                ct = p1.tile([H, S], FP32, tag="ct")
                st = p1.tile([H, S], FP32, tag="st")
                wv_sb = p1.tile([128, DT * KLOC * H], BF, tag="wvsb")

                for half in range(2):
                    scols = (half * (S // 2), (half + 1) * (S // 2))
                    xth_t = [p1.tile([128, 8 * (S // 2)], BF, tag="xth", bufs=4,
                                     name=f"xth{half}_{qq}") for qq in range(4)]

                    def xth_dma(qq, split=False):
                        base = (half * DT + qq * 8) * (S // 2)
                        if split:
                            hw_ = 4 * (S // 2)
                            nc.sync.dma_start(xth_t[qq][:, :hw_],
                                              xP.ap()[:, base: base + hw_])
                            nc.sync.dma_start(xth_t[qq][:, hw_:],
                                              xP.ap()[:, base + hw_: base + 8 * (S // 2)])
                        else:
                            nc.sync.dma_start(
                                xth_t[qq][:],
                                xP.ap()[:, base: base + 8 * (S // 2)])

                    if half == 1:
                        for qq in range(4):
                            xth_dma(qq)

                    def xth(d, a, b):
                        return xth_t[d // 8][:, (d % 8) * (S // 2) + a:
                                             (d % 8) * (S // 2) + b]

                    # q (8 head-tiles) then k (KLOC head-tiles); d-outer so one
                    # LDWEIGHTS covers two 512-wide matmuls.
                    for h in range(HEADS + KLOC):
                        wsrc = wq.ap()[h * 128:(h + 1) * 128, :] if h < HEADS \
                            else wk.ap()[(h - HEADS) * 128:(h - HEADS + 1) * 128, :]
                        if half == 0 and h == 0:
                            xth_dma(0, split=True)
                        wslab = p1.tile([128, DT * 128], BF, tag="wslab", bufs=2)
                        nc.sync.dma_start(wslab[:], wsrc)
                        if half == 0 and h == 0:
                            nc.sync.dma_start(ct[:], cosT.ap())
                            nc.sync.dma_start(st[:], sinST.ap())
                            for qq in range(1, 4):
                                xth_dma(qq)
                            nc.sync.dma_start(wv_sb[:], wv.ap())
                        qp = [p1ps.tile([128, 512], FP32, tag=f"qp{sc}", bufs=2,
                                        name=f"qp{half}_{h}_{sc}")
                              for sc in range(2)]
                        for d in range(DT):
                            for sc in range(2):
                                nc.tensor.matmul(
                                    qp[sc][:],
                                    lhsT=wslab[:, d * 128:(d + 1) * 128],
                                    rhs=xth(d, sc * 512, sc * 512 + 512),
                                    start=(d == 0), stop=(d == DT - 1))
                        for sc in range(2):
                            # rope: out = qp*cos + rot(qp)*sin_signed
                            gcol = scols[0] + sc * 512
                            t1 = p1.tile([128, 512], FP32, tag="t1", bufs=2)
                            nc.vector.tensor_mul(t1[:], qp[sc][:], ct[:, gcol:gcol + 512])
                            t2 = p1.tile([128, 512], FP32, tag="t2", bufs=2)
                            nc.vector.tensor_mul(t2[0:64, :], qp[sc][64:128, :],
                                                 st[0:64, gcol:gcol + 512])
                            nc.vector.tensor_mul(t2[64:128, :], qp[sc][0:64, :],
                                                 st[64:128, gcol:gcol + 512])
                            if h < HEADS:
                                robf = p1.tile([128, 512], BF, tag="robf", bufs=2)
                                nc.vector.tensor_add(robf[:], t1[:], t2[:])
                                nc.sync.dma_start(
                                    qT_dram[h * 128:(h + 1) * 128, gcol:gcol + 512],
                                    robf[:])
                            else:
                                nc.vector.tensor_add(
                                    kT_t[h - HEADS][:, gcol:gcol + 512], t1[:], t2[:])

                    # v projection for the 8 s-tiles of this half
                    for stl in range(ST // 2):
                        sti = half * (ST // 2) + stl
                        vp = p1ps.tile([128, KLOC * H], FP32, tag="vp", bufs=2)
                        for d in range(DT):
                            nc.tensor.matmul(
                                vp[:],
                                lhsT=xth(d, stl * 128, (stl + 1) * 128),
                                rhs=wv_sb[:, d * KLOC * H:(d + 1) * KLOC * H],
                                start=(d == 0), stop=(d == DT - 1))
                        nc.scalar.copy(v_t[sti][:], vp[:])

                for i in range(HEADS):
                    nc.sync.dma_start(wo_sb[i][:], wo.ap()[i * 128:(i + 1) * 128, :])

            # ---------------- Phase 2: attention + out-proj ----------------
            with tc.tile_pool(name="p2", bufs=1) as p2, \
                 tc.tile_pool(name="p2ps", bufs=1, space="PSUM") as p2ps:
                ones_sb = p2.tile([128, 128], BF, tag="ones")
                nc.gpsimd.memset(ones_sb[:], 1.0)
                if causal:
                    mdT_sb = p2.tile([128, S], FP32, tag="mdT")
                    nc.sync.dma_start(mdT_sb[:], mdT.ap())

                qg_all = [p2.tile([128, S], BF, tag=f"qga{h}", name=f"qga{h}")
                          for h in range(HEADS)]
                for h in range(HEADS):
                    nc.sync.dma_start(qg_all[h][:], qT_dram[h * 128:(h + 1) * 128, :])

                yts = {}       # (g, h) -> normalized yT tile in SBUF
                pending = []   # deferred per-head softmax finishers

                def finisher(g, h, kv, pts, yT_ps):
                    def fin():
                        jlast = len(pts) - 1
                        sums = p2ps.tile([1, 512], FP32, tag="sums", bufs=1)
                        for idx, (j, o, pt) in enumerate(pts):
                            nc.tensor.matmul(
                                sums[0:1, o:512], lhsT=ones_sb[:, 0:1],
                                rhs=pt[:, o:512],
                                start=(idx == 0), stop=(idx == jlast))
                        rc = p2.tile([1, 512], BF, tag="rc", bufs=2)
                        with nc.allow_low_precision(reason="1/Z broadcast in bf16"):
                            nc.vector.reciprocal(rc[0:1, :], sums[0:1, :])
                        rb = p2ps.tile([128, 512], FP32, tag="rb", bufs=1)
                        nc.tensor.matmul(rb[:], lhsT=ones_sb[0:1, :],
                                         rhs=rc[0:1, :], start=True, stop=True)
                        rbs = p2.tile([128, 512], BF, tag="rbs", bufs=2)
                        nc.scalar.copy(rbs[:], rb[:])
                        yt = p2.tile([128, 512], BF, tag=f"yts{h}", bufs=2,
                                     name=f"yts{g}_{h}")
                        nc.vector.tensor_mul(yt[:], yT_ps[:], rbs[:])
                        yts[(g, h)] = yt
                    return fin

                for g in range(NG):
                    q0 = g * 512
                    jmax = 4 * g + 3 if causal else ST - 1
                    if not causal:
                        mk_t = [p2.tile([128, 512], FP32, tag=f"mk{j}",
                                        name=f"mk{g}_{j}") for j in range(ST)]
                        for j in range(ST):
                            nc.sync.dma_start(
                                mk_t[j][:],
                                maskTf.ap()[j * 128:(j + 1) * 128, q0:q0 + 512])

                    for h in range(HEADS):
                        kv = h % KLOC
                        pts = []
                        yT_ps = p2ps.tile([128, 512], FP32, tag="yT", bufs=2)
                        prev = None
                        for j in range(jmax + 1):
                            o = max(0, j - 4 * g) * 128 if causal else 0
                            sps = p2ps.tile([128, 512], FP32, tag="sps", bufs=2)
                            nc.tensor.matmul(
                                sps[:, o:512],
                                lhsT=kT_t[kv][:, j * 128:(j + 1) * 128],
                                rhs=qg_all[h][:, q0 + o:q0 + 512],
                                start=True, stop=True)
                            if j == 2 and pending:
                                pending.pop(0)()
                            if causal:
                                if j >= 4 * g:  # diagonal block: i == j
                                    nc.vector.tensor_add(
                                        sps[:, o:o + 128], sps[:, o:o + 128],
                                        mdT_sb[:, j * 128:(j + 1) * 128])
                            else:
                                nc.vector.tensor_add(sps[:], sps[:], mk_t[j][:])
                            pt = p2.tile([128, 512], BF, tag="pt", bufs=20)
                            nc.scalar.activation(
                                pt[:, o:512], sps[:, o:512],
                                mybir.ActivationFunctionType.Exp)
                            if prev is not None:
                                pj, po, ppt = prev
                                nc.tensor.matmul(
                                    yT_ps[:, po:512],
                                    lhsT=v_t[pj][:, kv * H:(kv + 1) * H],
                                    rhs=ppt[:, po:512],
                                    start=(pj == 0), stop=False)
                            pts.append((j, o, pt))
                            prev = (j, o, pt)
                        pj, po, ppt = prev
                        nc.tensor.matmul(
                            yT_ps[:, po:512],
                            lhsT=v_t[pj][:, kv * H:(kv + 1) * H],
                            rhs=ppt[:, po:512],
                            start=(pj == 0), stop=True)
                        pending.append(finisher(g, h, kv, pts, yT_ps))

                    while pending:
                        pending.pop(0)()

                    # out-projection for this strip; RS per cc chunk
                    for it in range(4):
                        i = 4 * g + it
                        cg = next(ci for ci, (cs, cn) in enumerate(CC_CHUNKS)
                                  if cs <= i < cs + cn)
                        c_start, c_n = CC_CHUNKS[cg]
                        for dcg in range(4):
                            ops = [p2ps.tile([128, 512], FP32, tag=f"op{d2}",
                                             bufs=1, name=f"op{i}_{dcg}_{d2}")
                                   for d2 in range(2)]
                            for hh in range(HEADS):
                                for d2 in range(2):
                                    dc = dcg * 2 + d2
                                    nc.tensor.matmul(
                                        ops[d2][:],
                                        lhsT=yts[(g, hh)][:, it * 128:(it + 1) * 128],
                                        rhs=wo_sb[hh][:, dc * 512:(dc + 1) * 512],
                                        start=(hh == 0), stop=(hh == HEADS - 1))
                            for d2 in range(2):
                                dc = dcg * 2 + d2
                                oev = p2.tile([128, 512], BF, tag="oev", bufs=16)
                                if (dcg + d2) % 2:
                                    nc.scalar.copy(oev[:], ops[d2][:])
                                else:
                                    nc.vector.tensor_copy(oev[:], ops[d2][:])
                                nc.sync.dma_start(
                                    cc_in[cg][(i - c_start) * 128:(i - c_start + 1) * 128,
                                              dc * 512:(dc + 1) * 512],
                                    oev[:])
                        if i == c_start + c_n - 1:
                            nc.gpsimd.collective_compute(
                                "ReduceScatter", mybir.AluOpType.add,
                                replica_groups=[[0, 1, 2, 3], [4, 5, 6, 7]],
                                ins=[cc_in[cg].opt()], outs=[cc_out[cg].opt()])
                            orow = sum(cn * 32 for cs, cn in CC_CHUNKS[:cg])
                            nc.sync.dma_start(
                                out_sh.ap()[orow: orow + c_n * 32, :], cc_out[cg][:])

    nc.compile()
    return nc


_CANON_MASK = None


def _is_causal(mask: np.ndarray) -> bool:
    global _CANON_MASK
    if _CANON_MASK is None:
        _CANON_MASK = np.triu(np.full((S, S), -1e9, dtype=np.float32), k=1)
    return mask.shape == (S, S) and np.array_equal(mask, _CANON_MASK)


def _prepare(x, wq, wk, wv, wo, mask, sin, cos):
    causal = _is_causal(np.asarray(mask, dtype=np.float32))
    if causal not in _CACHE:
        _CACHE[causal] = _build(causal)
    nc = _CACHE[causal]

    x = np.asarray(x, dtype=np.float32)
    scale = np.float32(H ** -0.5)
    cosT = np.ascontiguousarray(np.asarray(cos, np.float32).T)          # [H, S]
    sinT = np.asarray(sin, np.float32).T.copy()                          # [H, S]
    sinT[0:H // 2] = -sinT[0:H // 2]                                     # signed
    # per-core weight shards; head order = r-major over local kv heads
    in_maps = []
    for c in range(N_CORES):
        b, tp = c // TP, c % TP
        ks = slice(tp * KLOC, (tp + 1) * KLOC)
        wq_c = np.asarray(wq, np.float32)[:, :, ks, :].reshape(D, HEADS * H)
        wk_c = (np.asarray(wk, np.float32)[:, ks, :] * scale).reshape(D, KLOC * H)
        wv_c = np.asarray(wv, np.float32)[:, ks, :].reshape(D, KLOC * H)
        m = {
            "xP": x[b].reshape(2, S // 2, DT, 128).transpose(3, 0, 2, 1)
                     .reshape(128, 2 * DT * (S // 2)).astype(BF16),
            "wq": wq_c.reshape(DT, 128, HEADS, H).transpose(2, 1, 0, 3)
                      .reshape(HEADS * 128, DT * 128).astype(BF16),
            "wk": wk_c.reshape(DT, 128, KLOC, H).transpose(2, 1, 0, 3)
                      .reshape(KLOC * 128, DT * 128).astype(BF16),
            "wv": wv_c.reshape(DT, 128, KLOC * H).transpose(1, 0, 2)
                      .reshape(128, DT * KLOC * H).astype(BF16),
            "wo": np.asarray(wo, np.float32)[:, ks, :, :].reshape(HEADS * H, D).astype(BF16),
            "cosT": cosT,
            "sinST": sinT,
        }
        if causal:
            md = np.empty((128, S), np.float32)
            for i in range(ST):
                md[:, i * 128:(i + 1) * 128] = \
                    mask[i * 128:(i + 1) * 128, i * 128:(i + 1) * 128].T
            m["mdT"] = md
        else:
            m["maskTf"] = np.ascontiguousarray(np.asarray(mask, np.float32).T)
        in_maps.append(m)
    return nc, in_maps


def _assemble(results):
    out = np.empty((B, S, D), dtype=np.float32)
    for c in range(N_CORES):
        b, tp = c // TP, c % TP
        sh = results[c]["out_shard"].astype(np.float32)
        orow = 0
        for cs, cn in CC_CHUNKS:
            rows = cn * 32
            out[b, cs * 128 + tp * rows: cs * 128 + (tp + 1) * rows, :] = \
                sh[orow: orow + rows]
            orow += rows
    return out


def kernel(x, wq, wk, wv, wo, mask, sin, cos):
    nc, in_maps = _prepare(x, wq, wk, wv, wo, mask, sin, cos)
    try:
        res = bass_utils.run_bass_kernel_spmd(nc, in_maps,
                                              core_ids=list(range(N_CORES)))
    except Exception:
        # transient device-side failures (e.g. NRT exec-unit errors) have
        # been observed once; a clean re-run succeeds.
        import time as _time
        _time.sleep(2.0)
        res = bass_utils.run_bass_kernel_spmd(nc, in_maps,
                                              core_ids=list(range(N_CORES)))
    return _assemble(res.results)


def _traced_run(x, wq, wk, wv, wo, mask, sin, cos):
    """Like kernel() but with NTFF tracing; returns BassKernelResults."""
    nc, in_maps = _prepare(x, wq, wk, wv, wo, mask, sin, cos)
    res = bass_utils.run_bass_kernel_spmd(nc, in_maps, core_ids=list(range(N_CORES)),
                                          trace=True)
    res.full_output = _assemble(res.results)
    return res


# revision 9
# speedup vs baseline: 1.0027x; 1.0027x over previous
"""Tensor-parallel fused attention kernel for Trainium2 (8 NeuronCores).

Sharding: DP=2 over batch x TP=4 over kv-head pairs. Each core computes
q/k/v projections + RoPE + causal attention + output projection for its
(batch, 2 kv heads) shard in bf16, then a 4-core ReduceScatter combines
the partial output projections; the host assembles the disjoint row
shards into the full [2, 2048, 4096] output.

Attention is computed in transposed-score layout: sT[kv, q] comes
straight out of matmul(lhsT=kT_j, rhs=qT), exp(sT) feeds the yT
accumulation directly (no per-block transpose matmuls), softmax row
sums come from an accumulating ones-vector matmul over the saved
exp tiles, and 1/Z is broadcast with a rank-1 matmul and folded into
the yT PSUM evacuation.
"""
import sys

for _p in ("/opt/trn_rl_repo", "/root/.axon_site/_ro/trn_rl_repo"):
    if _p not in sys.path:
        sys.path.append(_p)

import math
import numpy as np
import ml_dtypes

import concourse.bass as bass
import concourse.mybir as mybir
import concourse.tile as tile
from concourse import bacc
from concourse import bass_utils

BF16 = ml_dtypes.bfloat16
FP32 = mybir.dt.float32
BF = mybir.dt.bfloat16

B, S, D = 2, 2048, 4096
R, K, H = 4, 8, 128
N_CORES = 8
TP = 4            # tensor-parallel ways (kv-head axis)
KLOC = K // TP    # kv heads per core = 2
HEADS = R * KLOC  # query heads per core = 8
DT = D // 128     # 32 d-tiles
ST = S // 128     # 16 s-tiles
NG = ST // 4      # 4 strips of 512 rows
# ReduceScatter chunks: (start_tile, n_tiles); last two are single-tile to
# shorten the serial tail after the final out-projection.
CC_CHUNKS = [(0, 2), (2, 2), (4, 2), (6, 2), (8, 2), (10, 2), (12, 2), (14, 1), (15, 1)]

_CACHE = {}


def _build(causal: bool):
    nc = bacc.Bacc("TRN2", target_bir_lowering=False, debug=False,
                   enable_asserts=False, num_devices=N_CORES)

    xP = nc.dram_tensor("xP", [128, 2 * DT * (S // 2)], BF, kind="ExternalInput")
    wq = nc.dram_tensor("wq", [HEADS * 128, DT * 128], BF, kind="ExternalInput")
    wk = nc.dram_tensor("wk", [KLOC * 128, DT * 128], BF, kind="ExternalInput")
    wv = nc.dram_tensor("wv", [128, DT * KLOC * H], BF, kind="ExternalInput")
    wo = nc.dram_tensor("wo", [HEADS * H, D], BF, kind="ExternalInput")
    cosT = nc.dram_tensor("cosT", [H, S], FP32, kind="ExternalInput")
    sinST = nc.dram_tensor("sinST", [H, S], FP32, kind="ExternalInput")
    if causal:
        mdT = nc.dram_tensor("mdT", [128, S], FP32, kind="ExternalInput")
    else:
        maskTf = nc.dram_tensor("maskTf", [S, S], FP32, kind="ExternalInput")
    out_sh = nc.dram_tensor("out_shard", [S // TP, D], BF, kind="ExternalOutput")

    with tile.TileContext(nc) as tc:
        with tc.tile_pool(name="persist", bufs=1) as persist, \
             tc.tile_pool(name="dram", bufs=1, space="DRAM") as dram:

            kT_t = [persist.tile([128, S], BF, tag=f"kT{i}", name=f"kT{i}")
                    for i in range(KLOC)]
            v_t = [persist.tile([128, KLOC * H], BF, tag=f"v{i}", name=f"v{i}")
                   for i in range(ST)]
            wo_sb = [persist.tile([128, D], BF, tag=f"wo{i}", name=f"wo{i}")
                     for i in range(HEADS)]
            qT_dram = dram.tile([HEADS * 128, S], BF, tag="qtd", name="qT_dram")
            cc_in = [dram.tile([n * 128, D], BF, tag=f"ccin{g}", name=f"cc_in{g}")
                     for g, (st0, n) in enumerate(CC_CHUNKS)]
            cc_out = [dram.tile([n * 32, D], BF, tag=f"ccout{g}", name=f"cc_out{g}")
                      for g, (st0, n) in enumerate(CC_CHUNKS)]

            # ---------------- Phase 1: projections + rope ----------------
            with tc.tile_pool(name="p1", bufs=1) as p1, \
                 tc.tile_pool(name="p1ps", bufs=1, space="PSUM") as p1ps:
                ct = p1.tile([H, S], FP32, tag="ct")
                st = p1.tile([H, S], FP32, tag="st")
                wv_sb = p1.tile([128, DT * KLOC * H], BF, tag="wvsb")

                for half in range(2):
                    scols = (half * (S // 2), (half + 1) * (S // 2))
                    xth_t = [p1.tile([128, 8 * (S // 2)], BF, tag="xth", bufs=4,
                                     name=f"xth{half}_{qq}") for qq in range(4)]

                    def xth_dma(qq, split=False):
                        base = (half * DT + qq * 8) * (S // 2)
                        if split:
                            hw_ = 4 * (S // 2)
                            nc.sync.dma_start(xth_t[qq][:, :hw_],
                                              xP.ap()[:, base: base + hw_])
                            nc.sync.dma_start(xth_t[qq][:, hw_:],
                                              xP.ap()[:, base + hw_: base + 8 * (S // 2)])
                        else:
                            nc.sync.dma_start(
                                xth_t[qq][:],
                                xP.ap()[:, base: base + 8 * (S // 2)])

                    if half == 1:
                        for qq in range(4):
                            xth_dma(qq)

                    def xth(d, a, b):
                        return xth_t[d // 8][:, (d % 8) * (S // 2) + a:
                                             (d % 8) * (S // 2) + b]

                    # q (8 head-tiles) then k (KLOC head-tiles); d-outer so one
                    # LDWEIGHTS covers two 512-wide matmuls.
                    for h in range(HEADS + KLOC):
                        wsrc = wq.ap()[h * 128:(h + 1) * 128, :] if h < HEADS \
                            else wk.ap()[(h - HEADS) * 128:(h - HEADS + 1) * 128, :]
                        if half == 0 and h == 0:
                            xth_dma(0, split=True)
                        wslab = p1.tile([128, DT * 128], BF, tag="wslab", bufs=2)
                        nc.sync.dma_start(wslab[:], wsrc)
                        if half == 0 and h == 0:
                            nc.sync.dma_start(ct[:], cosT.ap())
                            nc.sync.dma_start(st[:], sinST.ap())
                            for qq in range(1, 4):
                                xth_dma(qq)
                            nc.sync.dma_start(wv_sb[:], wv.ap())
                        qp = [p1ps.tile([128, 512], FP32, tag=f"qp{sc}", bufs=2,
                                        name=f"qp{half}_{h}_{sc}")
                              for sc in range(2)]
                        for d in range(DT):
                            for sc in range(2):
                                nc.tensor.matmul(
                                    qp[sc][:],
                                    lhsT=wslab[:, d * 128:(d + 1) * 128],
                                    rhs=xth(d, sc * 512, sc * 512 + 512),
                                    start=(d == 0), stop=(d == DT - 1))
                        for sc in range(2):
                            # rope: out = qp*cos + rot(qp)*sin_signed
                            gcol = scols[0] + sc * 512
                            t1 = p1.tile([128, 512], FP32, tag="t1", bufs=2)
                            nc.vector.tensor_mul(t1[:], qp[sc][:], ct[:, gcol:gcol + 512])
                            t2 = p1.tile([128, 512], FP32, tag="t2", bufs=2)
                            nc.vector.tensor_mul(t2[0:64, :], qp[sc][64:128, :],
                                                 st[0:64, gcol:gcol + 512])
                            nc.vector.tensor_mul(t2[64:128, :], qp[sc][0:64, :],
                                                 st[64:128, gcol:gcol + 512])
                            if h < HEADS:
                                robf = p1.tile([128, 512], BF, tag="robf", bufs=2)
                                nc.vector.tensor_add(robf[:], t1[:], t2[:])
                                nc.sync.dma_start(
                                    qT_dram[h * 128:(h + 1) * 128, gcol:gcol + 512],
                                    robf[:])
                            else:
                                nc.vector.tensor_add(
                                    kT_t[h - HEADS][:, gcol:gcol + 512], t1[:], t2[:])

                    # v projection for the 8 s-tiles of this half
                    for stl in range(ST // 2):
                        sti = half * (ST // 2) + stl
                        vp = p1ps.tile([128, KLOC * H], FP32, tag="vp", bufs=2)
                        for d in range(DT):
                            nc.tensor.matmul(
                                vp[:],
                                lhsT=xth(d, stl * 128, (stl + 1) * 128),
                                rhs=wv_sb[:, d * KLOC * H:(d + 1) * KLOC * H],
                                start=(d == 0), stop=(d == DT - 1))
                        nc.scalar.copy(v_t[sti][:], vp[:])

                for i in range(HEADS):
                    nc.sync.dma_start(wo_sb[i][:], wo.ap()[i * 128:(i + 1) * 128, :])

            # ---------------- Phase 2: attention + out-proj ----------------
            with tc.tile_pool(name="p2", bufs=1) as p2, \
                 tc.tile_pool(name="p2ps", bufs=1, space="PSUM") as p2ps:
                ones_sb = p2.tile([128, 128], BF, tag="ones")
                nc.gpsimd.memset(ones_sb[:], 1.0)
                if causal:
                    mdT_sb = p2.tile([128, S], FP32, tag="mdT")
                    nc.sync.dma_start(mdT_sb[:], mdT.ap())

                qg_all = [p2.tile([128, S], BF, tag=f"qga{h}", name=f"qga{h}")
                          for h in range(HEADS)]
                for h in range(HEADS):
                    nc.sync.dma_start(qg_all[h][:], qT_dram[h * 128:(h + 1) * 128, :])

                yts = {}       # (g, h) -> normalized yT tile in SBUF
                pending = []   # deferred per-head softmax finishers

                def finisher(g, h, kv, zb_ps, yT_ps):
                    def fin():
                        # 1/Z = exp(-ln(Z)), broadcast across partitions for
                        # free by the ones-matmul that accumulated zb_ps.
                        lnz = p2.tile([128, 512], FP32, tag="lnz", bufs=2)
                        nc.scalar.activation(lnz[:], zb_ps[:],
                                             mybir.ActivationFunctionType.Ln)
                        rbs = p2.tile([128, 512], BF, tag="rbs", bufs=2)
                        nc.scalar.activation(rbs[:], lnz[:],
                                             mybir.ActivationFunctionType.Exp,
                                             scale=-1.0)
                        yt = p2.tile([128, 512], BF, tag=f"yts{h}", bufs=2,
                                     name=f"yts{g}_{h}")
                        nc.vector.tensor_mul(yt[:], yT_ps[:], rbs[:])
                        yts[(g, h)] = yt
                    return fin

                for g in range(NG):
                    q0 = g * 512
                    jmax = 4 * g + 3 if causal else ST - 1
                    if not causal:
                        mk_t = [p2.tile([128, 512], FP32, tag=f"mk{j}",
                                        name=f"mk{g}_{j}") for j in range(ST)]
                        for j in range(ST):
                            nc.sync.dma_start(
                                mk_t[j][:],
                                maskTf.ap()[j * 128:(j + 1) * 128, q0:q0 + 512])

                    for h in range(HEADS):
                        kv = h % KLOC
                        yT_ps = p2ps.tile([128, 512], FP32, tag="yT", bufs=2)
                        zb_ps = p2ps.tile([128, 512], FP32, tag="zb", bufs=2)
                        prev = None
                        for j in range(jmax + 1):
                            o = max(0, j - 4 * g) * 128 if causal else 0
                            sps = p2ps.tile([128, 512], FP32, tag="sps", bufs=2)
                            nc.tensor.matmul(
                                sps[:, o:512],
                                lhsT=kT_t[kv][:, j * 128:(j + 1) * 128],
                                rhs=qg_all[h][:, q0 + o:q0 + 512],
                                start=True, stop=True)
                            if j == 2 and pending:
                                pending.pop(0)()
                            if causal:
                                if j >= 4 * g:  # diagonal block: i == j
                                    nc.vector.tensor_add(
                                        sps[:, o:o + 128], sps[:, o:o + 128],
                                        mdT_sb[:, j * 128:(j + 1) * 128])
                            else:
                                nc.vector.tensor_add(sps[:], sps[:], mk_t[j][:])
                            pt = p2.tile([128, 512], BF, tag="pt", bufs=6)
                            nc.scalar.activation(
                                pt[:, o:512], sps[:, o:512],
                                mybir.ActivationFunctionType.Exp)
                            if prev is not None:
                                pj, po, ppt = prev
                                # Z accumulation: ones lhsT broadcasts the
                                # column sums across all 128 out partitions.
                                nc.tensor.matmul(
                                    zb_ps[:, po:512], lhsT=ones_sb[:, :],
                                    rhs=ppt[:, po:512],
                                    start=(pj == 0), stop=(pj == jmax))
                                nc.tensor.matmul(
                                    yT_ps[:, po:512],
                                    lhsT=v_t[pj][:, kv * H:(kv + 1) * H],
                                    rhs=ppt[:, po:512],
                                    start=(pj == 0), stop=False)
                            prev = (j, o, pt)
                        pj, po, ppt = prev
                        nc.tensor.matmul(
                            zb_ps[:, po:512], lhsT=ones_sb[:, :],
                            rhs=ppt[:, po:512],
                            start=(pj == 0), stop=(pj == jmax))
                        nc.tensor.matmul(
                            yT_ps[:, po:512],
                            lhsT=v_t[pj][:, kv * H:(kv + 1) * H],
                            rhs=ppt[:, po:512],
                            start=(pj == 0), stop=True)
                        pending.append(finisher(g, h, kv, zb_ps, yT_ps))

                    while pending:
                        pending.pop(0)()

                    # out-projection for this strip; RS per cc chunk
                    for it in range(4):
                        i = 4 * g + it
                        cg = next(ci for ci, (cs, cn) in enumerate(CC_CHUNKS)
                                  if cs <= i < cs + cn)
                        c_start, c_n = CC_CHUNKS[cg]
                        for dcg in range(4):
                            ops = [p2ps.tile([128, 512], FP32, tag=f"op{d2}",
                                             bufs=1, name=f"op{i}_{dcg}_{d2}")
                                   for d2 in range(2)]
                            for hh in range(HEADS):
                                for d2 in range(2):
                                    dc = dcg * 2 + d2
                                    nc.tensor.matmul(
                                        ops[d2][:],
                                        lhsT=yts[(g, hh)][:, it * 128:(it + 1) * 128],
                                        rhs=wo_sb[hh][:, dc * 512:(dc + 1) * 512],
                                        start=(hh == 0), stop=(hh == HEADS - 1))
                            for d2 in range(2):
                                dc = dcg * 2 + d2
                                oev = p2.tile([128, 512], BF, tag="oev", bufs=16)
                                if (dcg + d2) % 2:
                                    nc.scalar.copy(oev[:], ops[d2][:])
                                else:
                                    nc.vector.tensor_copy(oev[:], ops[d2][:])
                                nc.sync.dma_start(
                                    cc_in[cg][(i - c_start) * 128:(i - c_start + 1) * 128,
                                              dc * 512:(dc + 1) * 512],
                                    oev[:])
                        if i == c_start + c_n - 1:
                            nc.gpsimd.collective_compute(
                                "ReduceScatter", mybir.AluOpType.add,
                                replica_groups=[[0, 1, 2, 3], [4, 5, 6, 7]],
                                ins=[cc_in[cg].opt()], outs=[cc_out[cg].opt()])
                            orow = sum(cn * 32 for cs, cn in CC_CHUNKS[:cg])
                            nc.sync.dma_start(
                                out_sh.ap()[orow: orow + c_n * 32, :], cc_out[cg][:])

    nc.compile()
    return nc


_CANON_MASK = None


def _is_causal(mask: np.ndarray) -> bool:
    global _CANON_MASK
    if _CANON_MASK is None:
        _CANON_MASK = np.triu(np.full((S, S), -1e9, dtype=np.float32), k=1)
    return mask.shape == (S, S) and np.array_equal(mask, _CANON_MASK)


def _prepare(x, wq, wk, wv, wo, mask, sin, cos):
    causal = _is_causal(np.asarray(mask, dtype=np.float32))
    if causal not in _CACHE:
        _CACHE[causal] = _build(causal)
    nc = _CACHE[causal]

    x = np.asarray(x, dtype=np.float32)
    scale = np.float32(H ** -0.5)
    cosT = np.ascontiguousarray(np.asarray(cos, np.float32).T)          # [H, S]
    sinT = np.asarray(sin, np.float32).T.copy()                          # [H, S]
    sinT[0:H // 2] = -sinT[0:H // 2]                                     # signed
    # per-core weight shards; head order = r-major over local kv heads
    in_maps = []
    for c in range(N_CORES):
        b, tp = c // TP, c % TP
        ks = slice(tp * KLOC, (tp + 1) * KLOC)
        wq_c = np.asarray(wq, np.float32)[:, :, ks, :].reshape(D, HEADS * H)
        wk_c = (np.asarray(wk, np.float32)[:, ks, :] * scale).reshape(D, KLOC * H)
        wv_c = np.asarray(wv, np.float32)[:, ks, :].reshape(D, KLOC * H)
        m = {
            "xP": x[b].reshape(2, S // 2, DT, 128).transpose(3, 0, 2, 1)
                     .reshape(128, 2 * DT * (S // 2)).astype(BF16),
            "wq": wq_c.reshape(DT, 128, HEADS, H).transpose(2, 1, 0, 3)
                      .reshape(HEADS * 128, DT * 128).astype(BF16),
            "wk": wk_c.reshape(DT, 128, KLOC, H).transpose(2, 1, 0, 3)
                      .reshape(KLOC * 128, DT * 128).astype(BF16),
            "wv": wv_c.reshape(DT, 128, KLOC * H).transpose(1, 0, 2)
                      .reshape(128, DT * KLOC * H).astype(BF16),
            "wo": np.asarray(wo, np.float32)[:, ks, :, :].reshape(HEADS * H, D).astype(BF16),
            "cosT": cosT,
            "sinST": sinT,
        }
        if causal:
            md = np.empty((128, S), np.float32)
            for i in range(ST):
                md[:, i * 128:(i + 1) * 128] = \
                    mask[i * 128:(i + 1) * 128, i * 128:(i + 1) * 128].T
            m["mdT"] = md
        else:
            m["maskTf"] = np.ascontiguousarray(np.asarray(mask, np.float32).T)
        in_maps.append(m)
    return nc, in_maps


def _assemble(results):
    out = np.empty((B, S, D), dtype=np.float32)
    for c in range(N_CORES):
        b, tp = c // TP, c % TP
        sh = results[c]["out_shard"].astype(np.float32)
        orow = 0
        for cs, cn in CC_CHUNKS:
            rows = cn * 32
            out[b, cs * 128 + tp * rows: cs * 128 + (tp + 1) * rows, :] = \
                sh[orow: orow + rows]
            orow += rows
    return out


def kernel(x, wq, wk, wv, wo, mask, sin, cos):
    nc, in_maps = _prepare(x, wq, wk, wv, wo, mask, sin, cos)
    try:
        res = bass_utils.run_bass_kernel_spmd(nc, in_maps,
                                              core_ids=list(range(N_CORES)))
    except Exception:
        # transient device-side failures (e.g. NRT exec-unit errors) have
        # been observed once; a clean re-run succeeds.
        import time as _time
        _time.sleep(2.0)
        res = bass_utils.run_bass_kernel_spmd(nc, in_maps,
                                              core_ids=list(range(N_CORES)))
    return _assemble(res.results)


def _traced_run(x, wq, wk, wv, wo, mask, sin, cos):
    """Like kernel() but with NTFF tracing; returns BassKernelResults."""
    nc, in_maps = _prepare(x, wq, wk, wv, wo, mask, sin, cos)
    res = bass_utils.run_bass_kernel_spmd(nc, in_maps, core_ids=list(range(N_CORES)),
                                          trace=True)
    res.full_output = _assemble(res.results)
    return res


# revision 10
# speedup vs baseline: 1.0620x; 1.0591x over previous
"""Tensor-parallel fused attention kernel for Trainium2 (8 NeuronCores).

Sharding: DP=2 over batch x TP=4 over kv-head pairs. Each core computes
q/k/v projections + RoPE + causal attention + output projection for its
(batch, 2 kv heads) shard in bf16, then a 4-core ReduceScatter combines
the partial output projections; the host assembles the disjoint row
shards into the full [2, 2048, 4096] output.

Attention is computed in transposed-score layout: sT[kv, q] comes
straight out of matmul(lhsT=kT_j, rhs=qT), exp(sT) feeds the yT
accumulation directly (no per-block transpose matmuls), softmax row
sums come from an accumulating ones-vector matmul over the saved
exp tiles, and 1/Z is broadcast with a rank-1 matmul and folded into
the yT PSUM evacuation.
"""
import sys

for _p in ("/opt/trn_rl_repo", "/root/.axon_site/_ro/trn_rl_repo"):
    if _p not in sys.path:
        sys.path.append(_p)

import math
import numpy as np
import ml_dtypes

import concourse.bass as bass
import concourse.mybir as mybir
import concourse.tile as tile
from concourse import bacc
from concourse import bass_utils

BF16 = ml_dtypes.bfloat16
FP32 = mybir.dt.float32
BF = mybir.dt.bfloat16

B, S, D = 2, 2048, 4096
R, K, H = 4, 8, 128
N_CORES = 8
TP = 4            # tensor-parallel ways (kv-head axis)
KLOC = K // TP    # kv heads per core = 2
HEADS = R * KLOC  # query heads per core = 8
DT = D // 128     # 32 d-tiles
ST = S // 128     # 16 s-tiles
NG = ST // 4      # 4 strips of 512 rows
# ReduceScatter chunks: (start_tile, n_tiles); last two are single-tile to
# shorten the serial tail after the final out-projection.
CC_CHUNKS = [(0, 2), (2, 2), (4, 2), (6, 2), (8, 2), (10, 2), (12, 2), (14, 1), (15, 1)]

_CACHE = {}


def _build(causal: bool):
    nc = bacc.Bacc("TRN2", target_bir_lowering=False, debug=False,
                   enable_asserts=False, num_devices=N_CORES)

    xP = nc.dram_tensor("xP", [128, 2 * DT * (S // 2)], BF, kind="ExternalInput")
    wq = nc.dram_tensor("wq", [HEADS * 128, DT * 128], BF, kind="ExternalInput")
    wk = nc.dram_tensor("wk", [KLOC * 128, DT * 128], BF, kind="ExternalInput")
    wv = nc.dram_tensor("wv", [128, DT * KLOC * H], BF, kind="ExternalInput")
    wo = nc.dram_tensor("wo", [HEADS * H, D], BF, kind="ExternalInput")
    cosT = nc.dram_tensor("cosT", [H, S], FP32, kind="ExternalInput")
    sinST = nc.dram_tensor("sinST", [H, S], FP32, kind="ExternalInput")
    if causal:
        mdT = nc.dram_tensor("mdT", [128, S], FP32, kind="ExternalInput")
    else:
        maskTf = nc.dram_tensor("maskTf", [S, S], FP32, kind="ExternalInput")
    out_sh = nc.dram_tensor("out_shard", [S // TP, D], BF, kind="ExternalOutput")

    with tile.TileContext(nc) as tc:
        with tc.tile_pool(name="persist", bufs=1) as persist, \
             tc.tile_pool(name="dram", bufs=1, space="DRAM") as dram:

            kT_t = [persist.tile([128, S], BF, tag=f"kT{i}", name=f"kT{i}")
                    for i in range(KLOC)]
            v_t = [persist.tile([128, KLOC * H], BF, tag=f"v{i}", name=f"v{i}")
                   for i in range(ST)]
            wo_sb = [persist.tile([128, D], BF, tag=f"wo{i}", name=f"wo{i}")
                     for i in range(HEADS)]
            qT_dram = dram.tile([HEADS * 128, S], BF, tag="qtd", name="qT_dram")
            cc_in = [dram.tile([n * 128, D], BF, tag=f"ccin{g}", name=f"cc_in{g}")
                     for g, (st0, n) in enumerate(CC_CHUNKS)]
            cc_out = [dram.tile([n * 32, D], BF, tag=f"ccout{g}", name=f"cc_out{g}")
                      for g, (st0, n) in enumerate(CC_CHUNKS)]

            # ---------------- Phase 1: projections + rope ----------------
            with tc.tile_pool(name="p1", bufs=1) as p1, \
                 tc.tile_pool(name="p1ps", bufs=1, space="PSUM") as p1ps:
                ct = p1.tile([H, S], FP32, tag="ct")
                st = p1.tile([H, S], FP32, tag="st")
                wv_sb = p1.tile([128, DT * KLOC * H], BF, tag="wvsb")

                for half in range(2):
                    scols = (half * (S // 2), (half + 1) * (S // 2))
                    xth_t = [p1.tile([128, 8 * (S // 2)], BF, tag="xth", bufs=4,
                                     name=f"xth{half}_{qq}") for qq in range(4)]

                    def xth_dma(qq, split=False):
                        base = (half * DT + qq * 8) * (S // 2)
                        if split:
                            hw_ = 4 * (S // 2)
                            nc.sync.dma_start(xth_t[qq][:, :hw_],
                                              xP.ap()[:, base: base + hw_])
                            nc.sync.dma_start(xth_t[qq][:, hw_:],
                                              xP.ap()[:, base + hw_: base + 8 * (S // 2)])
                        else:
                            nc.sync.dma_start(
                                xth_t[qq][:],
                                xP.ap()[:, base: base + 8 * (S // 2)])

                    if half == 1:
                        for qq in range(4):
                            xth_dma(qq)

                    def xth(d, a, b):
                        return xth_t[d // 8][:, (d % 8) * (S // 2) + a:
                                             (d % 8) * (S // 2) + b]

                    # q (8 head-tiles) then k (KLOC head-tiles); d-outer so one
                    # LDWEIGHTS covers two 512-wide matmuls.
                    for h in range(HEADS + KLOC):
                        wsrc = wq.ap()[h * 128:(h + 1) * 128, :] if h < HEADS \
                            else wk.ap()[(h - HEADS) * 128:(h - HEADS + 1) * 128, :]
                        if half == 0 and h == 0:
                            xth_dma(0, split=True)
                        wslab = p1.tile([128, DT * 128], BF, tag="wslab", bufs=2)
                        nc.sync.dma_start(wslab[:], wsrc)
                        if half == 0 and h == 0:
                            nc.sync.dma_start(ct[:], cosT.ap())
                            nc.sync.dma_start(st[:], sinST.ap())
                            for qq in range(1, 4):
                                xth_dma(qq)
                            nc.sync.dma_start(wv_sb[:], wv.ap())
                        qp = [p1ps.tile([128, 512], FP32, tag=f"qp{sc}", bufs=2,
                                        name=f"qp{half}_{h}_{sc}")
                              for sc in range(2)]
                        for d in range(DT):
                            for sc in range(2):
                                nc.tensor.matmul(
                                    qp[sc][:],
                                    lhsT=wslab[:, d * 128:(d + 1) * 128],
                                    rhs=xth(d, sc * 512, sc * 512 + 512),
                                    start=(d == 0), stop=(d == DT - 1))
                        for sc in range(2):
                            # rope: out = qp*cos + rot(qp)*sin_signed
                            gcol = scols[0] + sc * 512
                            t1 = p1.tile([128, 512], FP32, tag="t1", bufs=2)
                            nc.vector.tensor_mul(t1[:], qp[sc][:], ct[:, gcol:gcol + 512])
                            t2 = p1.tile([128, 512], FP32, tag="t2", bufs=2)
                            nc.vector.tensor_mul(t2[0:64, :], qp[sc][64:128, :],
                                                 st[0:64, gcol:gcol + 512])
                            nc.vector.tensor_mul(t2[64:128, :], qp[sc][0:64, :],
                                                 st[64:128, gcol:gcol + 512])
                            if h < HEADS:
                                robf = p1.tile([128, 512], BF, tag="robf", bufs=2)
                                nc.vector.tensor_add(robf[:], t1[:], t2[:])
                                nc.sync.dma_start(
                                    qT_dram[h * 128:(h + 1) * 128, gcol:gcol + 512],
                                    robf[:])
                            else:
                                nc.vector.tensor_add(
                                    kT_t[h - HEADS][:, gcol:gcol + 512], t1[:], t2[:])

                    # v projection for the 8 s-tiles of this half
                    for stl in range(ST // 2):
                        sti = half * (ST // 2) + stl
                        vp = p1ps.tile([128, KLOC * H], FP32, tag="vp", bufs=2)
                        for d in range(DT):
                            nc.tensor.matmul(
                                vp[:],
                                lhsT=xth(d, stl * 128, (stl + 1) * 128),
                                rhs=wv_sb[:, d * KLOC * H:(d + 1) * KLOC * H],
                                start=(d == 0), stop=(d == DT - 1))
                        nc.scalar.copy(v_t[sti][:], vp[:])

                for i in range(HEADS):
                    nc.sync.dma_start(wo_sb[i][:], wo.ap()[i * 128:(i + 1) * 128, :])

            # ---------------- Phase 2: attention + out-proj ----------------
            with tc.tile_pool(name="p2", bufs=1) as p2, \
                 tc.tile_pool(name="p2ps", bufs=1, space="PSUM") as p2ps:
                ones_sb = p2.tile([128, 128], BF, tag="ones")
                nc.gpsimd.memset(ones_sb[:], 1.0)
                if causal:
                    mdT_sb = p2.tile([128, S], FP32, tag="mdT")
                    nc.sync.dma_start(mdT_sb[:], mdT.ap())

                qg_all = [p2.tile([128, S], BF, tag=f"qga{h}", name=f"qga{h}")
                          for h in range(HEADS)]
                for h in range(HEADS):
                    nc.sync.dma_start(qg_all[h][:], qT_dram[h * 128:(h + 1) * 128, :])

                yts = {}       # (g, h) -> normalized yT tile in SBUF
                pending = []   # deferred per-head softmax finishers

                def finisher(g, h, kv, zb_ps, yT_ps):
                    def fin():
                        # 1/Z (already broadcast across partitions by the
                        # ones-matmul); single custom-DVE op, no ACT tables.
                        rbs = p2.tile([128, 512], FP32, tag="rbs", bufs=2)
                        nc.vector.reciprocal_approx_fast(rbs[:], zb_ps[:])
                        yt = p2.tile([128, 512], BF, tag=f"yts{h}", bufs=2,
                                     name=f"yts{g}_{h}")
                        nc.vector.tensor_mul(yt[:], yT_ps[:], rbs[:])
                        yts[(g, h)] = yt
                    return fin

                for g in range(NG):
                    q0 = g * 512
                    jmax = 4 * g + 3 if causal else ST - 1
                    if not causal:
                        mk_t = [p2.tile([128, 512], FP32, tag=f"mk{j}",
                                        name=f"mk{g}_{j}") for j in range(ST)]
                        for j in range(ST):
                            nc.sync.dma_start(
                                mk_t[j][:],
                                maskTf.ap()[j * 128:(j + 1) * 128, q0:q0 + 512])

                    for h in range(HEADS):
                        kv = h % KLOC
                        yT_ps = p2ps.tile([128, 512], FP32, tag="yT", bufs=2)
                        zb_ps = p2ps.tile([128, 512], FP32, tag="zb", bufs=2)
                        prev = None
                        for j in range(jmax + 1):
                            o = max(0, j - 4 * g) * 128 if causal else 0
                            sps = p2ps.tile([128, 512], FP32, tag="sps", bufs=2)
                            nc.tensor.matmul(
                                sps[:, o:512],
                                lhsT=kT_t[kv][:, j * 128:(j + 1) * 128],
                                rhs=qg_all[h][:, q0 + o:q0 + 512],
                                start=True, stop=True)
                            if j == 2 and pending:
                                pending.pop(0)()
                            if causal:
                                if j >= 4 * g:  # diagonal block: i == j
                                    nc.vector.tensor_add(
                                        sps[:, o:o + 128], sps[:, o:o + 128],
                                        mdT_sb[:, j * 128:(j + 1) * 128])
                            else:
                                nc.vector.tensor_add(sps[:], sps[:], mk_t[j][:])
                            pt = p2.tile([128, 512], BF, tag="pt", bufs=6)
                            nc.scalar.activation(
                                pt[:, o:512], sps[:, o:512],
                                mybir.ActivationFunctionType.Exp)
                            if prev is not None:
                                pj, po, ppt = prev
                                # Z accumulation: ones lhsT broadcasts the
                                # column sums across all 128 out partitions.
                                nc.tensor.matmul(
                                    zb_ps[:, po:512], lhsT=ones_sb[:, :],
                                    rhs=ppt[:, po:512],
                                    start=(pj == 0), stop=(pj == jmax))
                                nc.tensor.matmul(
                                    yT_ps[:, po:512],
                                    lhsT=v_t[pj][:, kv * H:(kv + 1) * H],
                                    rhs=ppt[:, po:512],
                                    start=(pj == 0), stop=False)
                            prev = (j, o, pt)
                        pj, po, ppt = prev
                        nc.tensor.matmul(
                            zb_ps[:, po:512], lhsT=ones_sb[:, :],
                            rhs=ppt[:, po:512],
                            start=(pj == 0), stop=(pj == jmax))
                        nc.tensor.matmul(
                            yT_ps[:, po:512],
                            lhsT=v_t[pj][:, kv * H:(kv + 1) * H],
                            rhs=ppt[:, po:512],
                            start=(pj == 0), stop=True)
                        pending.append(finisher(g, h, kv, zb_ps, yT_ps))

                    while pending:
                        pending.pop(0)()

                    # out-projection for this strip; RS per cc chunk
                    for it in range(4):
                        i = 4 * g + it
                        cg = next(ci for ci, (cs, cn) in enumerate(CC_CHUNKS)
                                  if cs <= i < cs + cn)
                        c_start, c_n = CC_CHUNKS[cg]
                        for dcg in range(4):
                            ops = [p2ps.tile([128, 512], FP32, tag=f"op{d2}",
                                             bufs=1, name=f"op{i}_{dcg}_{d2}")
                                   for d2 in range(2)]
                            for hh in range(HEADS):
                                for d2 in range(2):
                                    dc = dcg * 2 + d2
                                    nc.tensor.matmul(
                                        ops[d2][:],
                                        lhsT=yts[(g, hh)][:, it * 128:(it + 1) * 128],
                                        rhs=wo_sb[hh][:, dc * 512:(dc + 1) * 512],
                                        start=(hh == 0), stop=(hh == HEADS - 1))
                            for d2 in range(2):
                                dc = dcg * 2 + d2
                                oev = p2.tile([128, 512], BF, tag="oev", bufs=16)
                                if (dcg + d2) % 2:
                                    nc.scalar.copy(oev[:], ops[d2][:])
                                else:
                                    nc.vector.tensor_copy(oev[:], ops[d2][:])
                                nc.sync.dma_start(
                                    cc_in[cg][(i - c_start) * 128:(i - c_start + 1) * 128,
                                              dc * 512:(dc + 1) * 512],
                                    oev[:])
                        if i == c_start + c_n - 1:
                            nc.gpsimd.collective_compute(
                                "ReduceScatter", mybir.AluOpType.add,
                                replica_groups=[[0, 1, 2, 3], [4, 5, 6, 7]],
                                ins=[cc_in[cg].opt()], outs=[cc_out[cg].opt()])
                            orow = sum(cn * 32 for cs, cn in CC_CHUNKS[:cg])
                            nc.sync.dma_start(
                                out_sh.ap()[orow: orow + c_n * 32, :], cc_out[cg][:])

    nc.compile()
    return nc


_CANON_MASK = None


def _is_causal(mask: np.ndarray) -> bool:
    global _CANON_MASK
    if _CANON_MASK is None:
        _CANON_MASK = np.triu(np.full((S, S), -1e9, dtype=np.float32), k=1)
    return mask.shape == (S, S) and np.array_equal(mask, _CANON_MASK)


def _prepare(x, wq, wk, wv, wo, mask, sin, cos):
    causal = _is_causal(np.asarray(mask, dtype=np.float32))
    if causal not in _CACHE:
        _CACHE[causal] = _build(causal)
    nc = _CACHE[causal]

    x = np.asarray(x, dtype=np.float32)
    scale = np.float32(H ** -0.5)
    cosT = np.ascontiguousarray(np.asarray(cos, np.float32).T)          # [H, S]
    sinT = np.asarray(sin, np.float32).T.copy()                          # [H, S]
    sinT[0:H // 2] = -sinT[0:H // 2]                                     # signed
    # per-core weight shards; head order = r-major over local kv heads
    in_maps = []
    for c in range(N_CORES):
        b, tp = c // TP, c % TP
        ks = slice(tp * KLOC, (tp + 1) * KLOC)
        wq_c = np.asarray(wq, np.float32)[:, :, ks, :].reshape(D, HEADS * H)
        wk_c = (np.asarray(wk, np.float32)[:, ks, :] * scale).reshape(D, KLOC * H)
        wv_c = np.asarray(wv, np.float32)[:, ks, :].reshape(D, KLOC * H)
        m = {
            "xP": x[b].reshape(2, S // 2, DT, 128).transpose(3, 0, 2, 1)
                     .reshape(128, 2 * DT * (S // 2)).astype(BF16),
            "wq": wq_c.reshape(DT, 128, HEADS, H).transpose(2, 1, 0, 3)
                      .reshape(HEADS * 128, DT * 128).astype(BF16),
            "wk": wk_c.reshape(DT, 128, KLOC, H).transpose(2, 1, 0, 3)
                      .reshape(KLOC * 128, DT * 128).astype(BF16),
            "wv": wv_c.reshape(DT, 128, KLOC * H).transpose(1, 0, 2)
                      .reshape(128, DT * KLOC * H).astype(BF16),
            "wo": np.asarray(wo, np.float32)[:, ks, :, :].reshape(HEADS * H, D).astype(BF16),
            "cosT": cosT,
            "sinST": sinT,
        }
        if causal:
            md = np.empty((128, S), np.float32)
            for i in range(ST):
                md[:, i * 128:(i + 1) * 128] = \
                    mask[i * 128:(i + 1) * 128, i * 128:(i + 1) * 128].T
            m["mdT"] = md
        else:
            m["maskTf"] = np.ascontiguousarray(np.asarray(mask, np.float32).T)
        in_maps.append(m)
    return nc, in_maps


def _assemble(results):
    out = np.empty((B, S, D), dtype=np.float32)
    for c in range(N_CORES):
        b, tp = c // TP, c % TP
        sh = results[c]["out_shard"].astype(np.float32)
        orow = 0
        for cs, cn in CC_CHUNKS:
            rows = cn * 32
            out[b, cs * 128 + tp * rows: cs * 128 + (tp + 1) * rows, :] = \
                sh[orow: orow + rows]
            orow += rows
    return out


def kernel(x, wq, wk, wv, wo, mask, sin, cos):
    nc, in_maps = _prepare(x, wq, wk, wv, wo, mask, sin, cos)
    try:
        res = bass_utils.run_bass_kernel_spmd(nc, in_maps,
                                              core_ids=list(range(N_CORES)))
    except Exception:
        # transient device-side failures (e.g. NRT exec-unit errors) have
        # been observed once; a clean re-run succeeds.
        import time as _time
        _time.sleep(2.0)
        res = bass_utils.run_bass_kernel_spmd(nc, in_maps,
                                              core_ids=list(range(N_CORES)))
    return _assemble(res.results)


def _traced_run(x, wq, wk, wv, wo, mask, sin, cos):
    """Like kernel() but with NTFF tracing; returns BassKernelResults."""
    nc, in_maps = _prepare(x, wq, wk, wv, wo, mask, sin, cos)
    res = bass_utils.run_bass_kernel_spmd(nc, in_maps, core_ids=list(range(N_CORES)),
                                          trace=True)
    res.full_output = _assemble(res.results)
    return res


# revision 15
# speedup vs baseline: 1.1902x; 1.1207x over previous
"""Tensor-parallel fused attention kernel for Trainium2 (8 NeuronCores).

Sharding: DP=2 over batch x TP=4 over kv-head pairs. Each core computes
q/k/v projections + RoPE + causal attention + output projection for its
(batch, 2 kv heads) shard in bf16, then a 4-core ReduceScatter combines
the partial output projections; the host assembles the disjoint row
shards into the full [2, 2048, 4096] output.

Attention is computed in transposed-score layout: sT[kv, q] comes
straight out of matmul(lhsT=kT_j, rhs=qT), exp(sT) feeds the yT
accumulation directly (no per-block transpose matmuls), softmax row
sums come from an accumulating ones-vector matmul over the saved
exp tiles, and 1/Z is broadcast with a rank-1 matmul and folded into
the yT PSUM evacuation.
"""
import sys

for _p in ("/opt/trn_rl_repo", "/root/.axon_site/_ro/trn_rl_repo"):
    if _p not in sys.path:
        sys.path.append(_p)

import math
import numpy as np
import ml_dtypes

import concourse.bass as bass
import concourse.mybir as mybir
import concourse.tile as tile
from concourse import bacc
from concourse import bass_utils

BF16 = ml_dtypes.bfloat16
FP32 = mybir.dt.float32
BF = mybir.dt.bfloat16

B, S, D = 2, 2048, 4096
R, K, H = 4, 8, 128
N_CORES = 8
TP = 4            # tensor-parallel ways (kv-head axis)
KLOC = K // TP    # kv heads per core = 2
HEADS = R * KLOC  # query heads per core = 8
DT = D // 128     # 32 d-tiles
ST = S // 128     # 16 s-tiles
NG = ST // 4      # 4 strips of 512 rows
# ReduceScatter chunks: (start_tile, n_tiles); last two are single-tile to
# shorten the serial tail after the final out-projection.
CC_CHUNKS = [(0, 2), (2, 2), (4, 2), (6, 2), (8, 2), (10, 2),
             (12, 1), (13, 1), (14, 1), (15, 1)]

_CACHE = {}


def _build(causal: bool):
    nc = bacc.Bacc("TRN2", target_bir_lowering=False, debug=False,
                   enable_asserts=False, num_devices=N_CORES)

    xP = nc.dram_tensor("xP", [128, 2 * DT * (S // 2)], BF, kind="ExternalInput")
    wq = nc.dram_tensor("wq", [HEADS * 128, DT * 128], BF, kind="ExternalInput")
    wk = nc.dram_tensor("wk", [KLOC * 128, DT * 128], BF, kind="ExternalInput")
    wv = nc.dram_tensor("wv", [128, DT * KLOC * H], BF, kind="ExternalInput")
    wo = nc.dram_tensor("wo", [HEADS * H, D], BF, kind="ExternalInput")
    cosT = nc.dram_tensor("cosT", [H, S], FP32, kind="ExternalInput")
    sinST = nc.dram_tensor("sinST", [H, S], FP32, kind="ExternalInput")
    if causal:
        mdT = nc.dram_tensor("mdT", [128, S], FP32, kind="ExternalInput")
    else:
        maskTf = nc.dram_tensor("maskTf", [S, S], FP32, kind="ExternalInput")
    out_sh = nc.dram_tensor("out_shard", [S // TP, D], BF, kind="ExternalOutput")

    with tile.TileContext(nc) as tc:
        with tc.tile_pool(name="persist", bufs=1) as persist, \
             tc.tile_pool(name="dram", bufs=1, space="DRAM") as dram:

            kT_t = [persist.tile([128, S], BF, tag=f"kT{i}", name=f"kT{i}")
                    for i in range(KLOC)]
            v_t = [persist.tile([128, KLOC * H], BF, tag=f"v{i}", name=f"v{i}")
                   for i in range(ST)]
            wo_sb = [persist.tile([128, D], BF, tag=f"wo{i}", name=f"wo{i}")
                     for i in range(HEADS)]
            qT_dram = dram.tile([HEADS * 128, S], BF, tag="qtd", name="qT_dram")
            cc_in = [dram.tile([n * 128, D], BF, tag=f"ccin{g}", name=f"cc_in{g}")
                     for g, (st0, n) in enumerate(CC_CHUNKS)]
            cc_out = [dram.tile([n * 32, D], BF, tag=f"ccout{g}", name=f"cc_out{g}")
                      for g, (st0, n) in enumerate(CC_CHUNKS)]

            # ---------------- Phase 1: projections + rope ----------------
            with tc.tile_pool(name="p1", bufs=1) as p1, \
                 tc.tile_pool(name="p1ps", bufs=1, space="PSUM") as p1ps:
                ct = p1.tile([H, S], FP32, tag="ct")
                st = p1.tile([H, S], FP32, tag="st")
                wv_sb = p1.tile([128, DT * KLOC * H], BF, tag="wvsb")

                for half in range(2):
                    scols = (half * (S // 2), (half + 1) * (S // 2))
                    xth_t = [p1.tile([128, 8 * (S // 2)], BF, tag="xth", bufs=4,
                                     name=f"xth{half}_{qq}") for qq in range(4)]

                    def xth_dma(qq, split=False):
                        base = (half * DT + qq * 8) * (S // 2)
                        if split:
                            hw_ = 4 * (S // 2)
                            nc.sync.dma_start(xth_t[qq][:, :hw_],
                                              xP.ap()[:, base: base + hw_])
                            nc.sync.dma_start(xth_t[qq][:, hw_:],
                                              xP.ap()[:, base + hw_: base + 8 * (S // 2)])
                        else:
                            nc.sync.dma_start(
                                xth_t[qq][:],
                                xP.ap()[:, base: base + 8 * (S // 2)])

                    if half == 1:
                        for qq in range(4):
                            xth_dma(qq)

                    def xth(d, a, b):
                        return xth_t[d // 8][:, (d % 8) * (S // 2) + a:
                                             (d % 8) * (S // 2) + b]

                    # q (8 head-tiles) then k (KLOC head-tiles); d-outer so one
                    # LDWEIGHTS covers two 512-wide matmuls.
                    for h in range(HEADS + KLOC):
                        wsrc = wq.ap()[h * 128:(h + 1) * 128, :] if h < HEADS \
                            else wk.ap()[(h - HEADS) * 128:(h - HEADS + 1) * 128, :]
                        if half == 0 and h == 0:
                            xth_dma(0, split=True)
                        wslab = p1.tile([128, DT * 128], BF, tag="wslab", bufs=2)
                        if half == 0 and h == 0:
                            # finer chunks so the first matmul starts sooner
                            for wc in range(4):
                                nc.sync.dma_start(
                                    wslab[:, wc * 1024:(wc + 1) * 1024],
                                    wsrc[:, wc * 1024:(wc + 1) * 1024])
                        else:
                            nc.sync.dma_start(wslab[:], wsrc)
                        if half == 0 and h == 0:
                            nc.sync.dma_start(ct[:], cosT.ap())
                            nc.sync.dma_start(st[:], sinST.ap())
                            for qq in range(1, 4):
                                xth_dma(qq)
                            nc.sync.dma_start(wv_sb[:], wv.ap())
                        qp = [p1ps.tile([128, 512], FP32, tag=f"qp{sc}", bufs=2,
                                        name=f"qp{half}_{h}_{sc}")
                              for sc in range(2)]
                        for d in range(DT):
                            for sc in range(2):
                                nc.tensor.matmul(
                                    qp[sc][:],
                                    lhsT=wslab[:, d * 128:(d + 1) * 128],
                                    rhs=xth(d, sc * 512, sc * 512 + 512),
                                    start=(d == 0), stop=(d == DT - 1))
                        for sc in range(2):
                            # rope: out = qp*cos + rot(qp)*sin_signed
                            gcol = scols[0] + sc * 512
                            t1 = p1.tile([128, 512], FP32, tag="t1", bufs=2)
                            nc.vector.tensor_mul(t1[:], qp[sc][:], ct[:, gcol:gcol + 512])
                            t2 = p1.tile([128, 512], FP32, tag="t2", bufs=2)
                            nc.vector.tensor_mul(t2[0:64, :], qp[sc][64:128, :],
                                                 st[0:64, gcol:gcol + 512])
                            nc.vector.tensor_mul(t2[64:128, :], qp[sc][0:64, :],
                                                 st[64:128, gcol:gcol + 512])
                            if h < HEADS:
                                robf = p1.tile([128, 512], BF, tag="robf", bufs=2)
                                nc.vector.tensor_add(robf[:], t1[:], t2[:])
                                nc.sync.dma_start(
                                    qT_dram[h * 128:(h + 1) * 128, gcol:gcol + 512],
                                    robf[:])
                            else:
                                nc.vector.tensor_add(
                                    kT_t[h - HEADS][:, gcol:gcol + 512], t1[:], t2[:])

                    # v projection for the 8 s-tiles of this half
                    for stl in range(ST // 2):
                        sti = half * (ST // 2) + stl
                        vp = p1ps.tile([128, KLOC * H], FP32, tag="vp", bufs=2)
                        for d in range(DT):
                            nc.tensor.matmul(
                                vp[:],
                                lhsT=xth(d, stl * 128, (stl + 1) * 128),
                                rhs=wv_sb[:, d * KLOC * H:(d + 1) * KLOC * H],
                                start=(d == 0), stop=(d == DT - 1))
                        nc.scalar.copy(v_t[sti][:], vp[:])

                for i in range(HEADS):
                    nc.sync.dma_start(wo_sb[i][:], wo.ap()[i * 128:(i + 1) * 128, :])

            # ---------------- Phase 2: attention + out-proj ----------------
            with tc.tile_pool(name="p2", bufs=1) as p2, \
                 tc.tile_pool(name="p2ps", bufs=1, space="PSUM") as p2ps:
                ones_sb = p2.tile([128, 128], BF, tag="ones")
                nc.gpsimd.memset(ones_sb[:], 1.0)
                if causal:
                    mdT_sb = p2.tile([128, S], FP32, tag="mdT")
                    nc.sync.dma_start(mdT_sb[:], mdT.ap())

                qg_all = [p2.tile([128, S], BF, tag=f"qga{h}", name=f"qga{h}")
                          for h in range(HEADS)]
                for h in range(HEADS):
                    nc.sync.dma_start(qg_all[h][:], qT_dram[h * 128:(h + 1) * 128, :])

                yts = {}       # (g, h) -> normalized yT tile in SBUF
                pending = []   # deferred per-head softmax finishers
                op_queue = []  # deferred out-projection tile emitters

                def op_tile(g, i):
                    def emit():
                        cg = next(ci for ci, (cs, cn) in enumerate(CC_CHUNKS)
                                  if cs <= i < cs + cn)
                        c_start, c_n = CC_CHUNKS[cg]
                        it = i - 4 * g
                        for dcg in range(4):
                            ops = [p2ps.tile([128, 512], FP32, tag=f"op{d2}",
                                             bufs=1, name=f"op{i}_{dcg}_{d2}")
                                   for d2 in range(2)]
                            for hh in range(HEADS):
                                for d2 in range(2):
                                    dc = dcg * 2 + d2
                                    nc.tensor.matmul(
                                        ops[d2][:],
                                        lhsT=yts[(g, hh)][:, it * 128:(it + 1) * 128],
                                        rhs=wo_sb[hh][:, dc * 512:(dc + 1) * 512],
                                        start=(hh == 0), stop=(hh == HEADS - 1))
                            for d2 in range(2):
                                dc = dcg * 2 + d2
                                oev = p2.tile([128, 512], BF, tag="oev", bufs=16)
                                if (dcg + d2) % 2:
                                    nc.scalar.copy(oev[:], ops[d2][:])
                                else:
                                    nc.vector.tensor_copy(oev[:], ops[d2][:])
                                nc.sync.dma_start(
                                    cc_in[cg][(i - c_start) * 128:(i - c_start + 1) * 128,
                                              dc * 512:(dc + 1) * 512],
                                    oev[:])
                        if i == c_start + c_n - 1:
                            nc.gpsimd.collective_compute(
                                "ReduceScatter", mybir.AluOpType.add,
                                replica_groups=[[0, 1, 2, 3], [4, 5, 6, 7]],
                                ins=[cc_in[cg].opt()], outs=[cc_out[cg].opt()])
                            orow = sum(cn * 32 for cs, cn in CC_CHUNKS[:cg])
                            nc.sync.dma_start(
                                out_sh.ap()[orow: orow + c_n * 32, :], cc_out[cg][:])
                    return emit

                def finisher(g, h, kv, zb_ps, yT_ps):
                    def fin():
                        # 1/Z (already broadcast across partitions by the
                        # ones-matmul); single custom-DVE op, no ACT tables.
                        rbs = p2.tile([128, 512], FP32, tag="rbs", bufs=2)
                        nc.vector.reciprocal_approx_fast(rbs[:], zb_ps[:])
                        yt = p2.tile([128, 512], BF, tag=f"yts{h}", bufs=2,
                                     name=f"yts{g}_{h}")
                        nc.vector.tensor_mul(yt[:], yT_ps[:], rbs[:])
                        yts[(g, h)] = yt
                    return fin

                for g in range(NG):
                    q0 = g * 512
                    jmax = 4 * g + 3 if causal else ST - 1
                    if not causal:
                        mk_t = [p2.tile([128, 512], FP32, tag=f"mk{j}",
                                        name=f"mk{g}_{j}") for j in range(ST)]
                        for j in range(ST):
                            nc.sync.dma_start(
                                mk_t[j][:],
                                maskTf.ap()[j * 128:(j + 1) * 128, q0:q0 + 512])

                    for h in range(HEADS):
                        # spread previous strip's out-projection tiles across
                        # this strip's head loop so the ReduceScatter stream
                        # stays fed instead of bursting at strip boundaries.
                        if h % 2 == 0 and op_queue:
                            op_queue.pop(0)()
                        kv = h % KLOC
                        yT_ps = p2ps.tile([128, 512], FP32, tag="yT", bufs=2)
                        zb_ps = p2ps.tile([128, 512], FP32, tag="zb", bufs=2)
                        prev = None
                        for j in range(jmax + 1):
                            o = max(0, j - 4 * g) * 128 if causal else 0
                            sps = p2ps.tile([128, 512], FP32, tag="sps", bufs=2)
                            nc.tensor.matmul(
                                sps[:, o:512],
                                lhsT=kT_t[kv][:, j * 128:(j + 1) * 128],
                                rhs=qg_all[h][:, q0 + o:q0 + 512],
                                start=True, stop=True)
                            if j == 2 and pending:
                                pending.pop(0)()
                            if causal:
                                if j >= 4 * g:  # diagonal block: i == j
                                    nc.vector.tensor_add(
                                        sps[:, o:o + 128], sps[:, o:o + 128],
                                        mdT_sb[:, j * 128:(j + 1) * 128])
                            else:
                                nc.vector.tensor_add(sps[:], sps[:], mk_t[j][:])
                            pt = p2.tile([128, 512], BF, tag="pt", bufs=6)
                            nc.scalar.activation(
                                pt[:, o:512], sps[:, o:512],
                                mybir.ActivationFunctionType.Exp)
                            if prev is not None:
                                pj, po, ppt = prev
                                # Z accumulation: ones lhsT broadcasts the
                                # column sums across all 128 out partitions.
                                nc.tensor.matmul(
                                    zb_ps[:, po:512], lhsT=ones_sb[:, :],
                                    rhs=ppt[:, po:512],
                                    start=(pj == 0), stop=(pj == jmax))
                                nc.tensor.matmul(
                                    yT_ps[:, po:512],
                                    lhsT=v_t[pj][:, kv * H:(kv + 1) * H],
                                    rhs=ppt[:, po:512],
                                    start=(pj == 0), stop=False)
                            prev = (j, o, pt)
                        pj, po, ppt = prev
                        nc.tensor.matmul(
                            zb_ps[:, po:512], lhsT=ones_sb[:, :],
                            rhs=ppt[:, po:512],
                            start=(pj == 0), stop=(pj == jmax))
                        nc.tensor.matmul(
                            yT_ps[:, po:512],
                            lhsT=v_t[pj][:, kv * H:(kv + 1) * H],
                            rhs=ppt[:, po:512],
                            start=(pj == 0), stop=True)
                        pending.append(finisher(g, h, kv, zb_ps, yT_ps))

                    while pending:
                        pending.pop(0)()

                    # queue this strip's out-projection tiles; the last strip
                    # has no following attention, so emit its tiles now.
                    for it in range(4):
                        op_queue.append(op_tile(g, 4 * g + it))
                    if g == NG - 1:
                        while op_queue:
                            op_queue.pop(0)()

    nc.compile()
    return nc


_CANON_MASK = None


def _is_causal(mask: np.ndarray) -> bool:
    global _CANON_MASK
    if _CANON_MASK is None:
        _CANON_MASK = np.triu(np.full((S, S), -1e9, dtype=np.float32), k=1)
    return mask.shape == (S, S) and np.array_equal(mask, _CANON_MASK)


def _prepare(x, wq, wk, wv, wo, mask, sin, cos):
    causal = _is_causal(np.asarray(mask, dtype=np.float32))
    if causal not in _CACHE:
        _CACHE[causal] = _build(causal)
    nc = _CACHE[causal]

    x = np.asarray(x, dtype=np.float32)
    scale = np.float32(H ** -0.5)
    cosT = np.ascontiguousarray(np.asarray(cos, np.float32).T)          # [H, S]
    sinT = np.asarray(sin, np.float32).T.copy()                          # [H, S]
    sinT[0:H // 2] = -sinT[0:H // 2]                                     # signed
    # per-core weight shards; head order = r-major over local kv heads
    in_maps = []
    for c in range(N_CORES):
        b, tp = c // TP, c % TP
        ks = slice(tp * KLOC, (tp + 1) * KLOC)
        wq_c = np.asarray(wq, np.float32)[:, :, ks, :].reshape(D, HEADS * H)
        wk_c = (np.asarray(wk, np.float32)[:, ks, :] * scale).reshape(D, KLOC * H)
        wv_c = np.asarray(wv, np.float32)[:, ks, :].reshape(D, KLOC * H)
        m = {
            "xP": x[b].reshape(2, S // 2, DT, 128).transpose(3, 0, 2, 1)
                     .reshape(128, 2 * DT * (S // 2)).astype(BF16),
            "wq": wq_c.reshape(DT, 128, HEADS, H).transpose(2, 1, 0, 3)
                      .reshape(HEADS * 128, DT * 128).astype(BF16),
            "wk": wk_c.reshape(DT, 128, KLOC, H).transpose(2, 1, 0, 3)
                      .reshape(KLOC * 128, DT * 128).astype(BF16),
            "wv": wv_c.reshape(DT, 128, KLOC * H).transpose(1, 0, 2)
                      .reshape(128, DT * KLOC * H).astype(BF16),
            "wo": np.asarray(wo, np.float32)[:, ks, :, :].reshape(HEADS * H, D).astype(BF16),
            "cosT": cosT,
            "sinST": sinT,
        }
        if causal:
            md = np.empty((128, S), np.float32)
            for i in range(ST):
                md[:, i * 128:(i + 1) * 128] = \
                    mask[i * 128:(i + 1) * 128, i * 128:(i + 1) * 128].T
            m["mdT"] = md
        else:
            m["maskTf"] = np.ascontiguousarray(np.asarray(mask, np.float32).T)
        in_maps.append(m)
    return nc, in_maps


def _assemble(results):
    out = np.empty((B, S, D), dtype=np.float32)
    for c in range(N_CORES):
        b, tp = c // TP, c % TP
        sh = results[c]["out_shard"].astype(np.float32)
        orow = 0
        for cs, cn in CC_CHUNKS:
            rows = cn * 32
            out[b, cs * 128 + tp * rows: cs * 128 + (tp + 1) * rows, :] = \
                sh[orow: orow + rows]
            orow += rows
    return out


def kernel(x, wq, wk, wv, wo, mask, sin, cos):
    nc, in_maps = _prepare(x, wq, wk, wv, wo, mask, sin, cos)
    try:
        res = bass_utils.run_bass_kernel_spmd(nc, in_maps,
                                              core_ids=list(range(N_CORES)))
    except Exception:
        # transient device-side failures (e.g. NRT exec-unit errors) have
        # been observed once; a clean re-run succeeds.
        import time as _time
        _time.sleep(2.0)
        res = bass_utils.run_bass_kernel_spmd(nc, in_maps,
                                              core_ids=list(range(N_CORES)))
    return _assemble(res.results)


def _traced_run(x, wq, wk, wv, wo, mask, sin, cos):
    """Like kernel() but with NTFF tracing; returns BassKernelResults."""
    nc, in_maps = _prepare(x, wq, wk, wv, wo, mask, sin, cos)
    res = bass_utils.run_bass_kernel_spmd(nc, in_maps, core_ids=list(range(N_CORES)),
                                          trace=True)
    res.full_output = _assemble(res.results)
    return res
